# revision 22
# baseline (speedup 1.0000x reference)
"""Trainium2 Bass kernel for nn_AdaptATT: grouped directional-pooling attention.

Reference computation (per fused sample s in b*groups, cg=8 channels, 128x128):
  gx           : [s, c, h, w] input slice
  sig_h/sig_w  : sigmoid(w1 @ [row-means | col-means] + b1)
  gated        : gx * sig_h * sig_w
  x1           : per-channel GroupNorm(gated) * gn_w + gn_b
  x2           : conv3x3(gx, w3) + b3
  x11          : softmax_c(mean_pix(x1)) == softmax(gn_b)   (host-known!)
  x21          : softmax_c(mean_pix(x2))
  weights      : x11 . x2 + x21 . x1   (channel contraction)
  out          : gx * sigmoid(weights)

v2 design (per core: 2 blocks of 16 samples; partitions = (sample, channel);
free dim = row-padded pixels, one zero row above/below the image so every
conv tap reads valid zeros and needs no edge trimming):
  - x11.x2 folds into a 1-out-channel 3x3 conv; its 9 taps run as 5 fp8
    DoubleRow matmuls (2 taps per instruction, both k-tile slots read the
    same padded fp8 image at a constant offset delta). A separate fp8 copy
    of x ships from the host; bf16 copy remains for everything else.
  - GroupNorm affine folds into the x21 contraction: coefm2 = x21*rstd*gn_w
    block-diag lhsT, beta2 = beta + sum_c x21*bias_gn, so x1 is never
    materialized (no per-tile ACT).
  - mean/var of gated via ACT-engine accumulate (in-place Copy + chunked
    Square), not bn_stats: keeps DVE free.
  - column sums via DVE tree-folds (bf16 2x mode) instead of a strided
    reduce; row sums via a reduce into bf16.
  - weight-stationary tile groups of 4 amortize LDWEIGHTS and keep the PE
    ramped; broadcast matmuls trail one group behind so the PE never waits
    on ACT sigmoids. Block 1's stats chunks interleave into block 0's
    final-phase issue order to keep every engine busy.

Toolchain quirks: every TPB compute instruction gets at most ONE sync-wait
(walrus) via packed constants, engine warm-ups and a post-schedule pass that
spills extra waits onto InstNoOps. GpSimd is DMA-issue only.
"""

import sys

if "/opt/trn_rl_repo" not in sys.path:
    sys.path.insert(0, "/opt/trn_rl_repo")

import os
import numpy as np

K_STATS = os.environ.get("K_STATS", "bn")    # act | bn
K_CONV = os.environ.get("K_CONV", "fp8x9")   # dr | fp8x9
K_DRPAIRS = set(int(x) for x in os.environ.get(
    "K_DRPAIRS", "0,1,2,3,4").split(",") if x != "")

B, C, H, W = 8, 256, 128, 128
GROUPS = 32
CG = C // GROUPS           # 8 channels per group
EPS = 1e-5
N_CORES = 8
BG = B * GROUPS            # 256 fused samples
S_PER_CORE = BG // N_CORES  # 32
S_BLK = 16                 # samples per device block (16*8 = 128 partitions)
N_BLK = S_PER_CORE // S_BLK  # 2
HW = H * W                 # 16384
ROWS_T = 3                 # image rows per psum tile (contiguous 390 incl pads)
N_TILES = 43               # 43 tiles x 3 rows = 129 rows (row 128 is junk)
TILE_GROUP = 2             # tiles per weight-stationary group / out batch
PADW = W + 2               # padded row stride (2 left pad cols)
FREE_T = ROWS_T * PADW     # 390 psum cols per tile (incl 2 junk cols per row)
IMG0 = 2                   # lead pad; padded row r starts at IMG0 + r*PADW
NPIX = IMG0 + 131 * PADW + 2  # zero row, 128 image rows, 2 zero rows, tail

# conv tap pairs for DoubleRow (spatial offsets -1..1). HW quirk found by
# bisection: a DoubleRow whose slot delta is PADW must be the LAST DR in an
# accumulation chain — any further DR after it wedges the device. So the
# delta-PADW pair goes last; all other pairs use delta 1.
PAIRS = [
    (((-1, -1), (-1, 0))),   # delta 1
    (((0, -1), (0, 0))),     # delta 1
    (((1, -1), (1, 0))),     # delta 1
    (((1, 1), None)),        # delta 1 (dup slot, zero weights)
    (((-1, 1), (0, 1))),     # delta PADW — must stay last
]

# fp32 packed constants (free-dim offsets in [128, CPK_F])
OFF_W3T = 0            # [128, 9*128] block-diag w3.T per tap
OFF_SBLK = 1152        # [128, 16] block-diag ones (fp32)
OFF_B1T = 1168         # [128, 1]
OFF_B3T = 1169         # [128, 1]
OFF_GNW = 1170         # [128, 1]
OFF_GNB = 1171         # [128, 1]
OFF_B16 = 1172         # [16, 128] broadcast lhsT fp32 (rows 0-15)
OFF_BETA = 1300        # [16, 1]
CPK_F = 1301

# bf16 packed constants
OFFB_W1T = 0           # [128, 128] block-diag w1.T / 128
OFFB_B16 = 128         # [16, 128] broadcast lhsT bf16
OFFB_SBLK = 256        # [128, 16] block-diag ones bf16
CPKB_F = 272

# fp8 packed constants
OFF8_V = 0             # [128, 5*2*16] DoubleRow conv-v lhsT pairs
CPK8_F = 160

_CACHE = {}


def _build_nc(split=True):
    import concourse.bass as bass
    import concourse.tile as tile
    from concourse import mybir

    fp32 = mybir.dt.float32
    bf16 = mybir.dt.bfloat16
    fp8 = mybir.dt.float8e4
    AX = mybir.AxisListType
    ACT = mybir.ActivationFunctionType
    PM = mybir.MatmulPerfMode

    nc = bass.Bass()

    xb_d = nc.declare_dram_parameter("xb", [C, NPIX], bf16, isOutput=False)
    x8_d = nc.declare_dram_parameter("x8", [C, NPIX], fp8, isOutput=False)
    cpk_d = nc.declare_dram_parameter("cpk", [128, CPK_F], fp32, isOutput=False)
    cpkb_d = nc.declare_dram_parameter("cpkb", [128, CPKB_F], bf16,
                                       isOutput=False)
    cpk8_d = nc.declare_dram_parameter("cpk8", [128, CPK8_F], fp8,
                                       isOutput=False)
    out_d = nc.declare_dram_parameter("out", [C, HW], bf16, isOutput=True)

    with tile.TileContext(nc) as tc, nc.allow_low_precision(reason="perf"):
        with (
            tc.tile_pool(name="singles", bufs=1) as singles,
            tc.tile_pool(name="gxbp", bufs=2) as gxbp,
            tc.tile_pool(name="gx8p", bufs=2) as gx8p,
            tc.tile_pool(name="gatedp", bufs=2) as gatedp,
            tc.tile_pool(name="scr", bufs=1) as scrp,
            tc.tile_pool(name="small", bufs=2) as small,
            tc.tile_pool(name="sigp", bufs=9) as sigp,
            tc.tile_pool(name="p3sp", bufs=3) as p3sp,
            tc.tile_pool(name="outp", bufs=2) as outp,
            tc.tile_pool(name="ps_p2", bufs=5, space="PSUM") as ps_p2,
            tc.tile_pool(name="ps_p3", bufs=2, space="PSUM") as ps_p3,
            tc.tile_pool(name="ps_misc", bufs=1, space="PSUM") as ps_misc,
        ):
            # ---- constants: one DMA per packed tensor ----
            # cpk8 first: dual-fp8 LDWEIGHTS wants an aligned SBUF base
            cpk8 = singles.tile([128, CPK8_F], fp8)
            nc.sync.dma_start(cpk8[:], cpk8_d[:])
            cpk = singles.tile([128, CPK_F], fp32)
            nc.sync.dma_start(cpk[:], cpk_d[:])
            w3t = cpk[:, OFF_W3T:OFF_W3T + 9 * 128]
            sblk = cpk[:, OFF_SBLK:OFF_SBLK + S_BLK]
            b1t = cpk[:, OFF_B1T:OFF_B1T + 1]
            b3t = cpk[:, OFF_B3T:OFF_B3T + 1]
            gnwt = cpk[:, OFF_GNW:OFF_GNW + 1]
            gnbt = cpk[:, OFF_GNB:OFF_GNB + 1]
            b16 = cpk[0:S_BLK, OFF_B16:OFF_B16 + 128]
            betat = cpk[0:S_BLK, OFF_BETA:OFF_BETA + 1]
            cpkb = singles.tile([128, CPKB_F], bf16)
            nc.sync.dma_start(cpkb[:], cpkb_d[:])
            w1tb = cpkb[:, OFFB_W1T:OFFB_W1T + 128]
            b16b = cpkb[0:S_BLK, OFFB_B16:OFFB_B16 + 128]
            sblkb = cpkb[:, OFFB_SBLK:OFFB_SBLK + S_BLK]
            epst = singles.tile([128, 1], fp32)
            nc.vector.memset(epst[:], EPS)
            # engine warm-ups: absorb const-DMA sem ticks
            p_wu = ps_misc.tile([1, 1], fp32, tag="m")
            nc.tensor.matmul(p_wu[:], cpk[:, 0:1], cpk[:, 0:1])
            p_wub = ps_misc.tile([1, 1], fp32, tag="m")
            nc.tensor.matmul(p_wub[:], cpkb[:, 0:1], cpkb[:, 0:1])
            p_wu8 = ps_misc.tile([16, 8], fp32, tag="m")
            nc.tensor.matmul(
                p_wu8[:],
                bass.AP(tensor=cpk8[:].tensor, offset=cpk8[:].offset,
                        ap=[[cpk8[:].ap[0][0], 128], [16, 2], [1, 16]]),
                bass.AP(tensor=cpk8[:].tensor, offset=cpk8[:].offset,
                        ap=[[cpk8[:].ap[0][0], 128], [1, 2], [1, 8]]),
                perf_mode=PM.DoubleRow)
            act_wu = singles.tile([128, 1], fp32)
            nc.scalar.copy(act_wu[:], cpk[:, 0:1])
            dve_wu = singles.tile([128, 1], fp32)
            nc.vector.tensor_copy(dve_wu[:], cpk[:, 0:1])

            # ---------------- per-block state ----------------
            blocks = [dict() for _ in range(N_BLK)]

            def load_block(blk):
                st = blocks[blk]
                gxb = gxbp.tile([128, NPIX], bf16)
                nc.gpsimd.dma_start(gxb[:], xb_d[blk * 128:(blk + 1) * 128, :])
                gx8 = gx8p.tile([128, NPIX], fp8)
                nc.gpsimd.dma_start(gx8[:], x8_d[blk * 128:(blk + 1) * 128, :])
                st["gxb"], st["gx8"] = gxb, gx8
                st["gxba"], st["gx8a"] = gxb[:], gx8[:]
                # image view [128, H, W] (rows 1..128, cols 2..129)
                st["gxb3"] = bass.AP(
                    tensor=st["gxba"].tensor,
                    offset=st["gxba"].offset + IMG0 + PADW + 2,
                    ap=[[st["gxba"].ap[0][0], 128], [PADW, H], [1, W]])

            def stats_chunks(blk):
                """Yield closures emitting one chunk of block-stats work."""
                st = blocks[blk]
                gxba = st["gxba"]
                gxb3 = st["gxb3"]

                def rowview(r0, nr):
                    # padded row r (image row r-1); incl 2 zero pad cols
                    return bass.AP(
                        tensor=gxba.tensor,
                        offset=gxba.offset + IMG0 + r0 * PADW,
                        ap=[[gxba.ap[0][0], 128], [PADW, nr], [1, PADW]])

                pooled = small.tile([128, H + PADW], bf16, tag="pooled")
                st["pooled"] = pooled
                cs = scrp.tile([128, 32 * PADW], bf16, tag="cs")
                cs3 = cs[:].rearrange("p (r q) -> p r q", q=PADW)

                # row sums (incl zero pad cols), 4 chunks of 32 rows
                def rs_chunk(i):
                    def f():
                        nc.vector.reduce_sum(
                            pooled[:, i * 32:(i + 1) * 32],
                            rowview(1 + i * 32, 32), axis=AX.X)
                    return f
                for i in range(4):
                    yield rs_chunk(i)

                # col sums: fold 128 rows -> 32 (seq) -> tree to 1
                def cs_fold(i):
                    def f():
                        if i == 0:
                            nc.vector.tensor_add(cs3, rowview(1, 32),
                                                 rowview(33, 32))
                        else:
                            nc.vector.tensor_add(cs3, cs3,
                                                 rowview(1 + 32 * (i + 1), 32))
                    return f
                for i in range(3):
                    yield cs_fold(i)

                def cs_tree():
                    n = 16
                    while n >= 1:
                        a = cs[:].rearrange("p (r q) -> p r q", q=PADW)
                        nc.vector.tensor_add(a[:, 0:n, :], a[:, 0:n, :],
                                             a[:, n:2 * n, :])
                        n //= 2
                    nc.vector.tensor_copy(pooled[:, H:H + PADW],
                                          cs[:, 0:PADW])
                yield cs_tree

                def mix():
                    p_hw = ps_misc.tile([128, H + PADW], fp32, tag="m", name="p_hw")
                    nc.tensor.matmul(p_hw[:], w1tb, pooled[:])
                    sig_hw = small.tile([128, H + PADW], bf16, tag="sighw")
                    nc.scalar.activation(sig_hw[:], p_hw[:], ACT.Sigmoid,
                                         bias=b1t)
                    st["sig_hw"] = sig_hw
                yield mix

                # gating: gated = (gx * sw) * sh, 4 chunks of 32 rows each
                # mul. Row-padded to stride PADW (+1 junk row for tile 42)
                # so the x21 matmul rhs is contiguous; pad cols are junk and
                # never read (junk psum cols are discarded downstream).
                gated = gatedp.tile([128, 129 * PADW], bf16)
                st["gated"] = gated
                st["gpad"] = gated
                g3 = bass.AP(
                    tensor=gated[:].tensor, offset=gated[:].offset + 2,
                    ap=[[gated[:].ap[0][0], 128], [PADW, H], [1, W]])
                def gzero():
                    # zero the junk row 128 and the per-row pad cols (both
                    # are read as junk psum cols by the x21 matmul)
                    nc.vector.memset(gated[:, 128 * PADW:129 * PADW], 0.0)
                    padcols = bass.AP(
                        tensor=gated[:].tensor, offset=gated[:].offset,
                        ap=[[gated[:].ap[0][0], 128], [PADW, 128], [1, 2]])
                    nc.vector.memset(padcols, 0.0)
                yield gzero

                def g1_chunk(i):
                    def f():
                        sw = st["sig_hw"][:, H + 2:H + 2 + W].unsqueeze(1) \
                            .to_broadcast([128, 32, W])
                        gv = bass.AP(
                            tensor=gxba.tensor,
                            offset=gxba.offset + IMG0 + (1 + 32 * i) * PADW + 2,
                            ap=[[gxba.ap[0][0], 128], [PADW, 32], [1, W]])
                        nc.vector.tensor_mul(g3[:, 32 * i:32 * (i + 1), :],
                                             gv, sw)
                    return f
                for i in range(4):
                    yield g1_chunk(i)

                def g2_chunk(i):
                    def f():
                        sh = st["sig_hw"][:, 32 * i:32 * (i + 1)] \
                            .unsqueeze(2).to_broadcast([128, 32, W])
                        seg = g3[:, 32 * i:32 * (i + 1), :]
                        nc.vector.tensor_mul(seg, seg, sh)
                    return f
                for i in range(4):
                    yield g2_chunk(i)

                # mean/var via ACT accumulate; runs on ACT so issue is cheap
                if K_STATS == "act":
                    def acc_sums():
                        gsum = small.tile([128, 1], fp32, tag="gsum")
                        gimg = bass.AP(
                            tensor=gated[:].tensor,
                            offset=gated[:].offset + 2,
                            ap=[[gated[:].ap[0][0], 128], [PADW, H], [1, W]])
                        nc.scalar.activation(gimg, gimg, ACT.Copy,
                                             accum_out=gsum[:])
                        sq4 = small.tile([128, 4], fp32, tag="sq4")
                        qs = cs[:].rearrange(
                            "p (r q) -> p r q", q=PADW)[:, 0:32, 0:W]
                        for i in range(4):
                            gseg = bass.AP(
                                tensor=gated[:].tensor,
                                offset=gated[:].offset + 2 + 32 * i * PADW,
                                ap=[[gated[:].ap[0][0], 128],
                                    [PADW, 32], [1, W]])
                            nc.scalar.activation(
                                qs, gseg, ACT.Square,
                                accum_out=sq4[:, i:i + 1])
                        st["gsum"], st["sq4"] = gsum, sq4
                    yield acc_sums
                else:
                    # stats over the contiguous padded region (rows 0..127
                    # incl zeroed pad cols); count corrected in gn_scalars
                    def acc_bn(i):
                        def f():
                            if i == 0:
                                st["bnst"] = small.tile([128, 33, 6], fp32,
                                                        tag="bnst",
                                                        name="bnst")
                            lo = i * 4 * 512
                            for k in range(4):
                                j = i * 4 + k
                                if j >= 33:
                                    continue
                                c0 = j * 512
                                c1 = min(c0 + 512, 128 * PADW)
                                nc.vector.bn_stats(st["bnst"][:, j, :],
                                                   gated[:, c0:c1])
                        return f
                    for i in range(9):
                        yield acc_bn(i)

                def gn_scalars():
                    mean = small.tile([128, 1], fp32, tag="mean")
                    vv = small.tile([128, 1], fp32, tag="vv")
                    if K_STATS == "act":
                        gsum, sq4 = st["gsum"], st["sq4"]
                        sq = small.tile([128, 1], fp32, tag="sq")
                        nc.vector.reduce_sum(sq[:], sq4[:], axis=AX.X)
                        nc.vector.tensor_scalar_mul(mean[:], gsum[:], 1.0 / HW)
                        msq = small.tile([128, 1], fp32, tag="msq")
                        nc.vector.tensor_mul(msq[:], mean[:], mean[:])
                        nc.vector.tensor_scalar_mul(vv[:], sq[:], 1.0 / HW)
                        nc.vector.tensor_sub(vv[:], vv[:], msq[:])
                    else:
                        mv = small.tile([128, 2], fp32, tag="mv")
                        nc.vector.bn_aggr(mv[:], st["bnst"][:])
                        # padded count correction: c = 16640/16384
                        cc = float(128 * PADW) / HW
                        nc.vector.tensor_scalar_mul(mean[:], mv[:, 0:1], cc)
                        msq2 = small.tile([128, 1], fp32, tag="msq2")
                        nc.vector.tensor_mul(msq2[:], mv[:, 0:1], mv[:, 0:1])
                        nc.vector.tensor_add(vv[:], mv[:, 1:2], msq2[:])
                        nc.vector.tensor_scalar_mul(vv[:], vv[:], cc)
                        nc.vector.tensor_mul(msq2[:], mean[:], mean[:])
                        nc.vector.tensor_sub(vv[:], vv[:], msq2[:])
                    sd = small.tile([128, 1], fp32, tag="sd")
                    nc.scalar.activation(sd[:], vv[:], ACT.Sqrt, bias=epst[:])
                    rstd = small.tile([128, 1], fp32, tag="rstd")
                    nc.vector.reciprocal(rstd[:], sd[:])
                    scale_gn = small.tile([128, 1], fp32, tag="scale_gn")
                    nc.vector.tensor_mul(scale_gn[:], rstd[:], gnwt)
                    mus = small.tile([128, 1], fp32, tag="mus")
                    nc.vector.tensor_mul(mus[:], mean[:], scale_gn[:])
                    bias_gn = small.tile([128, 1], fp32, tag="bias_gn")
                    nc.vector.tensor_sub(bias_gn[:], gnbt, mus[:])
                    st["scale_gn"], st["bias_gn"] = scale_gn, bias_gn
                yield gn_scalars

                def x2mean():
                    pooled_ = st["pooled"]
                    S_tot = small.tile([128, 1], fp32, tag="S_tot")
                    nc.vector.reduce_sum(S_tot[:], pooled_[:, 0:H], axis=AX.X)
                    corners = small.tile([128, 2, 2], fp32, tag="corners")
                    for ta, r in ((0, H - 1), (1, 0)):
                        for tb, cc in ((0, W - 1), (1, 0)):
                            nc.vector.tensor_copy(
                                corners[:, ta, tb:tb + 1],
                                st["gxb3"][:, r, cc:cc + 1])
                    t3a = small.tile([128, 3], fp32, tag="t3a")
                    nc.vector.tensor_sub(t3a[:, 0:1], S_tot[:],
                                         pooled_[:, H - 1:H])
                    nc.vector.tensor_copy(t3a[:, 1:2], S_tot[:])
                    nc.vector.tensor_sub(t3a[:, 2:3], S_tot[:],
                                         pooled_[:, 0:1])
                    c3 = small.tile([128, 3], fp32, tag="c3")
                    nc.vector.tensor_copy(c3[:, 0:1],
                                          pooled_[:, H + 2 + W - 1:H + 2 + W])
                    nc.vector.memset(c3[:, 1:2], 0.0)
                    nc.vector.tensor_copy(c3[:, 2:3], pooled_[:, H + 2:H + 3])
                    T9 = small.tile([128, 3, 3], fp32, tag="T9")
                    nc.vector.tensor_sub(
                        T9[:], t3a[:].unsqueeze(2).to_broadcast([128, 3, 3]),
                        c3[:].unsqueeze(1).to_broadcast([128, 3, 3]))
                    corn_view = T9[:, 0:3:2, 0:3:2]
                    nc.vector.tensor_add(corn_view, corn_view, corners[:])

                    p_m2 = ps_misc.tile([128, 1], fp32, tag="m", name="p_m2")
                    for ab in range(9):
                        nc.tensor.matmul(
                            p_m2[:], w3t[:, ab * 128:(ab + 1) * 128],
                            T9[:].rearrange("p a b -> p (a b)")[:, ab:ab + 1],
                            start=(ab == 0), stop=(ab == 8))
                    e8 = small.tile([128, 1], fp32, tag="e8")
                    nc.scalar.activation(e8[:], p_m2[:], ACT.Exp,
                                         bias=b3t, scale=1.0 / HW)
                    p_gs = ps_misc.tile([S_BLK, 1], fp32, tag="m", name="p_gs")
                    nc.tensor.matmul(p_gs[:], sblk, e8[:])
                    r16 = small.tile([S_BLK, 1], fp32, tag="r16")
                    nc.vector.reciprocal(r16[:], p_gs[:])
                    p_rb = ps_misc.tile([128, 1], fp32, tag="m", name="p_rb")
                    nc.tensor.matmul(p_rb[:], b16, r16[:])
                    rbs = small.tile([128, 1], fp32, tag="rbs")
                    nc.scalar.copy(rbs[:], p_rb[:])
                    x21c = small.tile([128, 1], fp32, tag="x21c")
                    nc.vector.tensor_mul(x21c[:], e8[:], rbs[:])
                    st["x21c"] = x21c
                yield x2mean

                def coef():
                    cs2 = small.tile([128, 1], fp32, tag="cs2")
                    nc.vector.tensor_mul(cs2[:], st["x21c"], st["scale_gn"][:])
                    coefm2 = small.tile([128, S_BLK], bf16, tag="coefm2")
                    nc.vector.tensor_mul(
                        coefm2[:], cs2[:].to_broadcast([128, S_BLK]), sblkb)
                    x21bg = small.tile([128, 1], fp32, tag="x21bg")
                    nc.vector.tensor_mul(x21bg[:], st["x21c"],
                                         st["bias_gn"][:])
                    p_b2 = ps_misc.tile([S_BLK, 1], fp32, tag="m", name="p_b2")
                    nc.tensor.matmul(p_b2[:], sblk, x21bg[:])
                    beta2 = small.tile([S_BLK, 1], fp32, tag="beta2")
                    nc.vector.tensor_add(beta2[:], p_b2[:], betat)
                    st["coefm2"], st["beta2"] = coefm2, beta2
                    # PE warm-up so the first x21 matmul needs one wait slot
                    p_wu2 = ps_misc.tile([S_BLK, 1], fp32, tag="m", name="p_wu2")
                    nc.tensor.matmul(p_wu2[:], coefm2[:], coefm2[:, 0:1])
                yield coef

            # ---------------- final phase ----------------
            def tiles_of(g):
                t0 = g * TILE_GROUP
                return list(range(t0, min(t0 + TILE_GROUP, N_TILES)))

            def emit_group_head(blk, g):
                """conv taps + x21 matmuls + sigmoids for group g."""
                st = blocks[blk]
                gx8a = st["gx8a"]
                tl = tiles_of(g)
                p2s, sigs = [], []
                for _ in tl:
                    p2s.append(ps_p2.tile([S_BLK, FREE_T], fp32,
                                          tag="p2", name="p2"))
                if K_CONV == "dr":
                    for q, (t1, t2) in enumerate(PAIRS):
                        a1, b1_ = t1
                        if t2 is None:
                            dlt = 1
                        else:
                            dlt = (t2[0] - a1) * PADW + (t2[1] - b1_)
                        if q in K_DRPAIRS:
                            lhs = bass.AP(
                                tensor=cpk8[:].tensor,
                                offset=cpk8[:].offset + OFF8_V + q * 32,
                                ap=[[cpk8[:].ap[0][0], 128], [16, 2],
                                    [1, 16]])
                            for i, t in enumerate(tl):
                                r0 = t * ROWS_T
                                base = IMG0 + (r0 + a1 + 1) * PADW + b1_
                                rhs = bass.AP(
                                    tensor=gx8a.tensor,
                                    offset=gx8a.offset + base,
                                    ap=[[gx8a.ap[0][0], 128], [dlt, 2],
                                        [1, FREE_T]])
                                nc.tensor.matmul(p2s[i][:], lhs, rhs,
                                                 start=(q == 0), stop=False,
                                                 perf_mode=PM.DoubleRow)
                        else:
                            for si, tp in enumerate((t1, t2)):
                                if tp is None:
                                    continue
                                a1s, b1s = tp
                                lhss = bass.AP(
                                    tensor=cpk8[:].tensor,
                                    offset=(cpk8[:].offset + OFF8_V + q * 32
                                            + si * 16),
                                    ap=[[cpk8[:].ap[0][0], 128], [1, 16]])
                                for i, t in enumerate(tl):
                                    r0 = t * ROWS_T
                                    base = (IMG0 + (r0 + a1s + 1) * PADW
                                            + b1s)
                                    rhs = bass.AP(
                                        tensor=gx8a.tensor,
                                        offset=gx8a.offset + base,
                                        ap=[[gx8a.ap[0][0], 128],
                                            [1, FREE_T]])
                                    nc.tensor.matmul(
                                        p2s[i][:], lhss, rhs,
                                        start=(q == 0 and si == 0),
                                        stop=False)
                else:
                    qi = 0
                    for q, (t1, t2) in enumerate(PAIRS):
                        for si, tp in enumerate((t1, t2)):
                            if tp is None:
                                continue
                            a1, b1_ = tp
                            lhs = bass.AP(
                                tensor=cpk8[:].tensor,
                                offset=(cpk8[:].offset + OFF8_V + q * 32
                                        + si * 16),
                                ap=[[cpk8[:].ap[0][0], 128], [1, 16]])
                            for i, t in enumerate(tl):
                                r0 = t * ROWS_T
                                base = IMG0 + (r0 + a1 + 1) * PADW + b1_
                                rhs = bass.AP(
                                    tensor=gx8a.tensor,
                                    offset=gx8a.offset + base,
                                    ap=[[gx8a.ap[0][0], 128], [1, FREE_T]])
                                nc.tensor.matmul(p2s[i][:], lhs, rhs,
                                                 start=(qi == 0), stop=False)
                            qi += 1
                gated = st["gated"]
                for i, t in enumerate(tl):
                    r0 = t * ROWS_T
                    # gated is row-padded to stride PADW with junk pad cols
                    nc.tensor.matmul(
                        p2s[i][:], st["coefm2"][:],
                        st["gpad"][:, r0 * PADW:r0 * PADW + FREE_T],
                        start=False, stop=True)
                for i, t in enumerate(tl):
                    sig = sigp.tile([S_BLK, FREE_T], bf16)
                    nc.scalar.activation(sig[:], p2s[i][:], ACT.Sigmoid,
                                         bias=st["beta2"][:])
                    sigs.append(sig)
                return sigs

            def emit_group_tail(blk, g, sigs):
                """broadcast + evict + final mul + out DMA for group g."""
                st = blocks[blk]
                gxba = st["gxba"]
                tl = tiles_of(g)
                ostage = outp.tile([128, TILE_GROUP * ROWS_T * W], bf16)
                for i, t in enumerate(tl):
                    p3 = ps_p3.tile([128, FREE_T], fp32)
                    nc.tensor.matmul(p3[:], b16b, sigs[i][:])
                    p3s = p3sp.tile([128, FREE_T], bf16)
                    nc.scalar.copy(p3s[:], p3[:])
                    r0 = t * ROWS_T
                    gv = bass.AP(
                        tensor=gxba.tensor,
                        offset=gxba.offset + IMG0 + (r0 + 1) * PADW + 2,
                        ap=[[gxba.ap[0][0], 128], [PADW, ROWS_T], [1, W]])
                    p3v = bass.AP(
                        tensor=p3s[:].tensor, offset=p3s[:].offset + 2,
                        ap=[[p3s[:].ap[0][0], 128], [PADW, ROWS_T], [1, W]])
                    oseg = ostage[:, i * ROWS_T * W:(i + 1) * ROWS_T * W]
                    nc.vector.tensor_mul(
                        oseg.rearrange("p (r c) -> p r c", r=ROWS_T), gv, p3v)
                # rows g*12 .. min(g*12+len(tl)*3, 128)
                row0 = g * TILE_GROUP * ROWS_T
                nrows = min(len(tl) * ROWS_T, H - row0)
                nc.gpsimd.dma_start(
                    out_d[blk * 128:(blk + 1) * 128,
                          row0 * W:(row0 + nrows) * W],
                    ostage[:, 0:nrows * W])

            # ---------------- schedule ----------------
            N_GROUPS = (N_TILES + TILE_GROUP - 1) // TILE_GROUP
            load_block(0)
            for f in stats_chunks(0):
                f()
            # block 0 final with block 1 load+stats interleaved
            load_block(1)
            chunks1 = list(stats_chunks(1))
            ci = 0
            per = (len(chunks1) + N_GROUPS - 1) // N_GROUPS
            pending = None
            for g in range(N_GROUPS):
                sigs = emit_group_head(0, g)
                if pending is not None:
                    emit_group_tail(0, g - 1, pending)
                pending = sigs
                for _ in range(per):
                    if ci < len(chunks1):
                        chunks1[ci]()
                        ci += 1
            while ci < len(chunks1):
                chunks1[ci]()
                ci += 1
            emit_group_tail(0, N_GROUPS - 1, pending)
            # block 1 final
            pending = None
            for g in range(N_GROUPS):
                sigs = emit_group_head(1, g)
                if pending is not None:
                    emit_group_tail(1, g - 1, pending)
                pending = sigs
            emit_group_tail(1, N_GROUPS - 1, pending)

    if split:
        _split_multi_waits(nc, mybir)
    return nc


# TPB compute instructions have a single HW sync-wait slot on this
# toolchain ("Too many sync wait commands" at walrus codegen otherwise).
_NO_SPLIT = {
    "InstEventSemaphore", "InstCall",
    "InstRegisterMove", "InstUnconditionalBranch", "InstTriggeredCopy",
}


def _split_multi_waits(nc, mybir):
    """Move all but one sync-wait of each compute instruction onto
    freshly inserted same-engine ENGINE_NOPs directly before it."""
    n = [0]

    def make_nop(engine, wait):
        n[0] += 1
        nop = mybir.InstNoOp(name=f"WSPLIT-{n[0]}", ins=[], outs=[],
                             engine=engine)
        nop.sync_info = mybir.SyncInfo(on_wait=[wait], on_update=[])
        return nop

    for bb in nc.m.functions[0].blocks:
        out = []
        for ins in bb.instructions:
            si = ins.sync_info
            waits = list(si.on_wait) if si is not None and si.on_wait else []
            if len(waits) > 1 and type(ins).__name__ not in _NO_SPLIT:
                for w in waits[:-1]:
                    out.append(make_nop(ins.engine, w))
                ins.sync_info = mybir.SyncInfo(on_wait=[waits[-1]],
                                               on_update=list(si.on_update))
            out.append(ins)
        bb.instructions[:] = out


def _host_constants(w1, b1, w3, b3, gn_w, gn_b):
    import ml_dtypes
    w1 = np.asarray(w1, np.float32)
    b1 = np.asarray(b1, np.float32)
    w3 = np.asarray(w3, np.float32)
    b3 = np.asarray(b3, np.float32)
    gn_w = np.asarray(gn_w, np.float32)
    gn_b = np.asarray(gn_b, np.float32)

    s = S_BLK
    cpk = np.zeros((128, CPK_F), np.float32)
    cpkb = np.zeros((128, CPKB_F), ml_dtypes.bfloat16)
    cpk8 = np.zeros((128, CPK8_F), ml_dtypes.float8_e4m3)

    # block-diag w1^T / W (bf16): lhsT[s*8+i, s*8+o] = w1[o, i] / 128
    for k in range(s):
        cpkb[k * CG:(k + 1) * CG,
             OFFB_W1T + k * CG:OFFB_W1T + (k + 1) * CG] = (
            w1.T / float(W)).astype(ml_dtypes.bfloat16)
    cpk[:, OFF_B1T] = np.tile(b1, s)

    # x11 = softmax(gn_b) (exact: x1 spatial mean == gn_b)
    eb = np.exp(gn_b - gn_b.max())
    x11 = (eb / eb.sum()).astype(np.float32)
    cpk[0:s, OFF_BETA] = float(np.dot(x11, b3))

    # v[c, a, b] = sum_o x11[o] * w3[o, c, a, b]; DoubleRow pair lhsT (fp8)
    v = np.einsum("o,ocab->cab", x11, w3).astype(np.float32)
    for q, (t1, t2) in enumerate(PAIRS):
        for i, tp in enumerate((t1, t2)):
            if tp is None:
                continue
            a, b = tp[0] + 1, tp[1] + 1
            col = np.asarray(v[:, a, b], np.float32)
            for k in range(s):
                cpk8[k * CG:(k + 1) * CG, OFF8_V + q * 32 + i * 16 + k] = (
                    col.astype(ml_dtypes.float8_e4m3))

    # w3 block-diag per tap (fp32, for the tiny mean-x2 matmuls)
    for ab in range(9):
        a, b = ab // 3, ab % 3
        for k in range(s):
            cpk[k * CG:(k + 1) * CG,
                OFF_W3T + ab * 128 + k * CG:
                OFF_W3T + ab * 128 + (k + 1) * CG] = w3[:, :, a, b].T
    cpk[:, OFF_B3T] = np.tile(b3, s)

    for k in range(s):
        cpk[k * CG:(k + 1) * CG, OFF_SBLK + k] = 1.0          # sblk
        cpk[k, OFF_B16 + k * CG:OFF_B16 + (k + 1) * CG] = 1.0  # b16
        cpkb[k * CG:(k + 1) * CG, OFFB_SBLK + k] = 1.0
        cpkb[k, OFFB_B16 + k * CG:OFFB_B16 + (k + 1) * CG] = 1.0

    cpk[:, OFF_GNW] = np.tile(gn_w, s)
    cpk[:, OFF_GNB] = np.tile(gn_b, s)

    return dict(cpk=cpk, cpkb=cpkb, cpk8=cpk8)


def _pad_shard(rows, dtype):
    """[C, HW] float rows -> [C, NPIX]: IMG0 lead pad, then 131 rows of
    stride PADW (zero row, 128 image rows, 2 zero rows), 2 left pad cols."""
    out = np.zeros((C, NPIX), dtype)
    out[:, IMG0:IMG0 + 131 * PADW].reshape(C, 131, PADW)[:, 1:H + 1, 2:] = (
        rows.reshape(C, H, W))
    return out


def _make_in_maps(x, w1, b1, w3, b3, gn_w, gn_b):
    import ml_dtypes
    consts = _host_constants(w1, b1, w3, b3, gn_w, gn_b)
    xv = np.asarray(x, np.float32).reshape(BG, CG, HW)
    in_maps = []
    for k in range(N_CORES):
        rows = xv[k * S_PER_CORE:(k + 1) * S_PER_CORE].reshape(C, HW)
        m = {"xb": _pad_shard(rows, ml_dtypes.bfloat16),
             "x8": _pad_shard(rows, ml_dtypes.float8_e4m3)}
        m.update(consts)
        in_maps.append(m)
    return in_maps


def kernel(x, w1, b1, w3, b3, gn_w, gn_b):
    from concourse.bass_utils import run_bass_kernel_spmd

    if "nc" not in _CACHE:
        _CACHE["nc"] = _build_nc()
    nc = _CACHE["nc"]

    in_maps = _make_in_maps(x, w1, b1, w3, b3, gn_w, gn_b)
    res = run_bass_kernel_spmd(nc, in_maps, core_ids=list(range(N_CORES)))
    outs = [np.asarray(res.results[k]["out"], np.float32)
            .reshape(S_PER_CORE, CG, H, W) for k in range(N_CORES)]
    return np.concatenate(outs, axis=0).reshape(B, C, H, W)
